# revision 8
# baseline (speedup 1.0000x reference)
"""3D bilateral filter (RADIUS=2, 5x5x5 window) on 8 Trainium2 NeuronCores.

Sharding: 8 cores = 2 batches x 4 z-slabs of 32 (halo 2 handled host-side).
Per-core layout: partitions = x (128), free dim = (z_local rows) x (padded y).
Out-of-volume taps are neutralized by padding with a large value BIG chosen so
the range weight exp(-c*(x-BIG)^2 + b) underflows to exactly 0 on the ACT LUT.
x-axis tap shifts are pre-materialized host-side as 5 shifted variants (plus a
second y-parity copy in fp16 mode, keeping DVE reads 4B-aligned for 2x mode).

Per tap on-chip:  D = x - x_shift (DVE), S = D^2 (ACT Square), W = exp(-c*S+b)
(ACT Exp, b = log spatial weight), P = W * x_shift (DVE), then num += P and
den += W via identity-matmul accumulation into PSUM (PE does all adds).
Finally out = num * reciprocal(den) (DVE) and DMA out.
"""

import os
import sys

import numpy as np

for _p in ("/root/.axon_site", "/root/.axon_site/_ro/trn_rl_repo",
           "/root/.axon_site/_ro/pypackages", "/opt/trn_rl_repo"):
    if os.path.isdir(_p) and _p not in sys.path:
        sys.path.append(_p)

import concourse.bacc as bacc
import concourse.mybir as mybir
from concourse.tile import TileContext
from concourse import bass_utils

RADIUS = 2
NTAPS = 5 * 5 * 5
X = 128  # partitions (dim 2 of input)
ZSLAB = 32  # output z rows per core
ZROWS = ZSLAB + 2 * RADIUS  # z rows incl halo
BLK = 16  # z rows per PSUM block
NBLK = ZSLAB // BLK

MODE = os.environ.get("BILAT_MODE", "f16")  # "f16" or "f32"
PAIRS = bool(int(os.environ.get("BILAT_PAIRS", "1")))  # pair-sharing kernel
TRACE = bool(int(os.environ.get("BILAT_TRACE", "0")))

LAST_RESULTS = None  # BassKernelResults of most recent run (for test.py)

_TAPS = [(dx, dy, dz)
         for dx in range(-RADIUS, RADIUS + 1)
         for dy in range(-RADIUS, RADIUS + 1)
         for dz in range(-RADIUS, RADIUS + 1)]

# canonical pair representatives: o lexicographically positive (dx in {0,1,2})
_PAIRS_O = [o for o in _TAPS if o > (0, 0, 0)]
_CLS_PATS = [(0, 0, 0)] + sorted({(abs(a), abs(b), abs(c)) for a, b, c in _PAIRS_O})
_CLS_IDX = {p: i for i, p in enumerate(_CLS_PATS)}
NCLS = len(_CLS_PATS)

_PROG_CACHE = {}


def _build_program(mode):
    f32 = mybir.dt.float32
    f32r = mybir.dt.float32r
    f16 = mybir.dt.float16
    if mode == "f16":
        dt_x, dt_wp, dt_id, nv, wid = f16, f16, f16, 10, 136
    else:
        dt_x, dt_wp, dt_id, nv, wid = f32, f32r, f32r, 5, 132
    np_x = mybir.dt.np(dt_x)

    nc = bacc.Bacc("TRN2", target_bir_lowering=False, debug=False, num_devices=8)
    xs = nc.dram_tensor("xs", [X, nv * ZROWS, wid], dt_x, kind="ExternalInput")
    cb = nc.dram_tensor("cb", [X, NTAPS + 1], f32, kind="ExternalInput")
    ident = nc.dram_tensor("ident", [X, X], f32, kind="ExternalInput")
    out = nc.dram_tensor("out", [X, ZSLAB * 128], f32, kind="ExternalOutput")

    Sq = mybir.ActivationFunctionType.Square
    Ex = mybir.ActivationFunctionType.Exp

    nb = 3 if mode == "f16" else 2
    with TileContext(nc) as tc:
        with (
            tc.tile_pool(name="big", bufs=1) as bigpool,
            tc.tile_pool(name="dd", bufs=nb) as dpool,
            tc.tile_pool(name="ss", bufs=nb) as spool,
            tc.tile_pool(name="ww", bufs=nb) as wpool,
            tc.tile_pool(name="pp", bufs=nb) as ppool,
            tc.tile_pool(name="ev", bufs=1) as epool,
            tc.tile_pool(name="ps", bufs=1, space="PSUM") as psp,
        ):
            xs_t = bigpool.tile([X, nv * ZROWS, wid], dt_x)
            nc.sync.dma_start(out=xs_t, in_=xs.ap())
            cb_t = bigpool.tile([X, NTAPS + 1], f32)
            nc.sync.dma_start(out=cb_t, in_=cb.ap())
            id_f32 = bigpool.tile([X, X], f32)
            nc.sync.dma_start(out=id_f32, in_=ident.ap())
            id_t = bigpool.tile([X, X], dt_id)
            nc.vector.tensor_copy(out=id_t, in_=id_f32)

            def read_ap(dx, dy, dz, blk):
                # AP into xs_t for tap (dx,dy,dz), z-block blk: [128,BLK,128]
                if mode == "f16":
                    v = (dx + RADIUS) * 2 + (dy & 1)
                    col0 = 2 + dy + (dy & 1)
                else:
                    v = dx + RADIUS
                    col0 = 2 + dy
                r0 = v * ZROWS + RADIUS + dz + BLK * blk
                return xs_t[:, r0 : r0 + BLK, col0 : col0 + 128]

            for blk in range(NBLK):
                p_num = psp.tile([X, BLK, 128], mybir.dt.float32, tag="num")
                p_den = psp.tile([X, BLK, 128], mybir.dt.float32, tag="den")
                for k, (dx, dy, dz) in enumerate(_TAPS):
                    first = k == 0
                    last = k == NTAPS - 1
                    base = read_ap(0, 0, 0, blk)
                    shft = read_ap(dx, dy, dz, blk)
                    d_t = dpool.tile([X, BLK, 128], dt_x)
                    nc.vector.tensor_sub(out=d_t, in0=base, in1=shft)
                    # Balance the square op between DVE (fp16 2x) and ACT
                    sq_on_dve = mode == "f16" and (k % 12) < 5
                    if sq_on_dve:
                        s_t = spool.tile([X, BLK, 128], dt_x, tag="s16")
                        nc.vector.tensor_mul(out=s_t, in0=d_t, in1=d_t)
                    else:
                        s_t = spool.tile([X, BLK, 128], mybir.dt.float32, tag="s32")
                        nc.scalar.activation(s_t, d_t, Sq)
                    w_t = wpool.tile([X, BLK, 128], dt_wp)
                    nc.scalar.activation(
                        w_t, s_t, Ex,
                        bias=cb_t[:, k : k + 1],
                        scale=cb_t[:, NTAPS : NTAPS + 1],
                    )
                    p_t = ppool.tile([X, BLK, 128], dt_wp)
                    nc.vector.tensor_mul(out=p_t, in0=w_t, in1=shft)
                    for r in range(BLK // 4):
                        nc.tensor.matmul(
                            p_num[:, 4 * r : 4 * r + 4, :], id_t,
                            p_t[:, 4 * r : 4 * r + 4, :],
                            start=first, stop=last,
                        )
                        nc.tensor.matmul(
                            p_den[:, 4 * r : 4 * r + 4, :], id_t,
                            w_t[:, 4 * r : 4 * r + 4, :],
                            start=first, stop=last,
                        )
                rec_t = epool.tile([X, BLK, 128], mybir.dt.float32, tag="rec")
                nc.vector.reciprocal(out=rec_t, in_=p_den)
                o_t = epool.tile([X, BLK, 128], mybir.dt.float32, tag="out")
                nc.vector.tensor_mul(out=o_t, in0=p_num, in1=rec_t)
                nc.sync.dma_start(
                    out=out.ap()[:, BLK * 128 * blk : BLK * 128 * (blk + 1)],
                    in_=o_t,
                )
    nc.compile()
    return nc, np_x


def _build_program_pairs():
    """fp16 pair kernel: one sub/square/exp per +-tap pair. The reverse tap's
    weight field is obtained by DMA-shifting the exp output across partitions
    (x) and free dims (z,y); spatial weights ride in scaled-identity lhsT."""
    f32 = mybir.dt.float32
    f16 = mybir.dt.float16
    WID = 136  # y at col 4, pads 4+4
    EC = 132  # ext-region cols (y' in [-2,130))
    ER = BLK + 4  # ext-region rows
    PZ = ZROWS + 4  # variant rows: z' = row - 4, rows 0,1,38,39 always pad

    nc = bacc.Bacc("TRN2", target_bir_lowering=False, debug=False, num_devices=8)
    xs = nc.dram_tensor("xs", [X, 6 * PZ, WID], f16, kind="ExternalInput")
    cbs = nc.dram_tensor("cbs", [X, 1], f32, kind="ExternalInput")
    wids = nc.dram_tensor("wids", [X, NCLS * 128], f16, kind="ExternalInput")
    out = nc.dram_tensor("out", [X, ZSLAB * 128], f32, kind="ExternalOutput")

    Sq = mybir.ActivationFunctionType.Square
    Ex = mybir.ActivationFunctionType.Exp

    with TileContext(nc) as tc:
        with (
            tc.tile_pool(name="big", bufs=1) as bigpool,
            tc.tile_pool(name="de", bufs=4) as depool,
            tc.tile_pool(name="p1", bufs=3) as p1pool,
            tc.tile_pool(name="es", bufs=4) as espool,
            tc.tile_pool(name="p2", bufs=3) as p2pool,
            tc.tile_pool(name="ev", bufs=1) as epool,
            tc.tile_pool(name="ps", bufs=1, space="PSUM") as psp,
        ):
            xs_t = bigpool.tile([X, 6 * PZ, WID], f16)
            nc.sync.dma_start(out=xs_t, in_=xs.ap())
            cbs_t = bigpool.tile([X, 1], f32)
            nc.sync.dma_start(out=cbs_t, in_=cbs.ap())
            wid_t = bigpool.tile([X, NCLS * 128], f16)
            nc.sync.dma_start(out=wid_t, in_=wids.ap())
            ones_t = bigpool.tile([X, BLK, 128], f16)
            nc.gpsimd.memset(ones_t, 1.0)

            def rd(v, r0, nr, c0, ncol):
                return xs_t[:, v * PZ + r0 : v * PZ + r0 + nr, c0 : c0 + ncol]

            def lhs(pat):
                c = _CLS_IDX[pat]
                return wid_t[:, c * 128 : (c + 1) * 128]

            for blk in range(NBLK):
                R0 = blk * BLK + 2  # ext rows = xs rows [R0, R0+ER)
                p_num = psp.tile([X, BLK, 128], f32, tag="num")
                p_den = psp.tile([X, BLK, 128], f32, tag="den")
                # center tap: W = 1
                for r in range(BLK // 4):
                    nc.tensor.matmul(
                        p_num[:, 4 * r : 4 * r + 4, :], lhs((0, 0, 0)),
                        rd(0, R0 + 2 + 4 * r, 4, 4, 128),
                        start=True, stop=False,
                    )
                    nc.tensor.matmul(
                        p_den[:, 4 * r : 4 * r + 4, :], lhs((0, 0, 0)),
                        ones_t[:, 4 * r : 4 * r + 4, :],
                        start=True, stop=False,
                    )
                for pi, (dx, dy, dz) in enumerate(_PAIRS_O):
                    last = pi == len(_PAIRS_O) - 1
                    v, vn = 3 + dx, 3 - dx
                    cls = lhs((dx, abs(dy), abs(dz)))
                    # D on ext region [ER x EC]
                    d_t = depool.tile([X, ER, EC], f16)
                    nc.vector.tensor_sub(
                        out=d_t,
                        in0=rd(0, R0, ER, 2, EC),
                        in1=rd(v, R0 + dz, ER, 2 + dy, EC),
                    )
                    # S = D^2 in place (split between DVE and ACT), E = exp(-c*S)
                    if pi % 7 < 2:
                        nc.vector.tensor_mul(out=d_t, in0=d_t, in1=d_t)
                    else:
                        nc.scalar.activation(d_t, d_t, Sq)
                    nc.scalar.activation(d_t, d_t, Ex, scale=cbs_t[:, 0:1])
                    # P1 = E * A_shift on base region
                    p1_t = p1pool.tile([X, BLK, 128], f16)
                    nc.vector.tensor_mul(
                        out=p1_t,
                        in0=d_t[:, 2 : 2 + BLK, 2:130],
                        in1=rd(v, R0 + 2 + dz, BLK, 4 + dy, 128),
                    )
                    # Es(j) = E(j - o): shifted copy via DMA (partitions + free)
                    es_t = espool.tile([X, BLK, 128], f16)
                    if dx > 0:
                        nc.gpsimd.memset(es_t[0:dx], 0.0)
                    for q in range(4):
                        nc.sync.dma_start(
                            out=es_t[dx:X, 4 * q : 4 * q + 4, :],
                            in_=d_t[
                                0 : X - dx,
                                (2 + 4 * q) - dz : (6 + 4 * q) - dz,
                                2 - dy : 130 - dy,
                            ],
                        )
                    # P2s = Es * A(j-o)
                    p2_t = p2pool.tile([X, BLK, 128], f16)
                    nc.vector.tensor_mul(
                        out=p2_t,
                        in0=es_t,
                        in1=rd(vn, R0 + 2 - dz, BLK, 4 - dy, 128),
                    )
                    for r in range(BLK // 4):
                        sl = slice(4 * r, 4 * r + 4)
                        nc.tensor.matmul(
                            p_num[:, sl, :], cls, p1_t[:, sl, :],
                            start=False, stop=False,
                        )
                        nc.tensor.matmul(
                            p_num[:, sl, :], cls, p2_t[:, sl, :],
                            start=False, stop=last,
                        )
                        nc.tensor.matmul(
                            p_den[:, sl, :], cls,
                            d_t[:, 2 + 4 * r : 6 + 4 * r, 2:130],
                            start=False, stop=False,
                        )
                        nc.tensor.matmul(
                            p_den[:, sl, :], cls, es_t[:, sl, :],
                            start=False, stop=last,
                        )
                rec_t = epool.tile([X, BLK, 128], f32, tag="rec")
                nc.vector.reciprocal(out=rec_t, in_=p_den)
                o_t = epool.tile([X, BLK, 128], f32, tag="out")
                nc.vector.tensor_mul(out=o_t, in0=p_num, in1=rec_t)
                nc.sync.dma_start(
                    out=out.ap()[:, BLK * 128 * blk : BLK * 128 * (blk + 1)],
                    in_=o_t,
                )
    nc.compile()
    return nc


def _prep_core_inputs_pairs(vol, z0, big):
    """Variants for the pair kernel: index 0 = base (+BIG pads), 1..5 = x-shift
    dx=-2..2 (-BIG pads). Width 136, y_real at col 4, z_local at row 2."""
    WID = 136
    PZ = ZROWS + 4
    data = np.empty((X, PZ, 128), np.float32)
    valid = np.zeros((PZ,), bool)
    zlo = z0 - 4  # row r holds z' = r - 4
    zs_lo, zs_hi = max(0, z0 - RADIUS), min(128, z0 + ZSLAB + RADIUS)
    data[:, zs_lo - zlo : zs_hi - zlo] = vol[:, :, zs_lo:zs_hi].transpose(0, 2, 1)
    valid[zs_lo - zlo : zs_hi - zlo] = True

    xs = np.empty((X, 6, PZ, WID), np.float32)
    xs[:, 0] = big
    xs[:, 0, valid, 4:132] = data[:, valid]
    for dx in range(-RADIUS, RADIUS + 1):
        vi = 3 + dx
        xs[:, vi] = -big
        if dx >= 0:
            xs[: X - dx, vi, valid, 4:132] = data[dx:][:, valid]
        else:
            xs[-dx:, vi, valid, 4:132] = data[: X + dx][:, valid]
    return xs.astype(np.float16).reshape(X, 6 * PZ, WID)


def _prep_core_inputs(vol, z0, big, np_x, mode):
    """vol: (128,128,128) f32 volume (x,y,z) for one batch. Returns xs array."""
    nv = 10 if mode == "f16" else 5
    wid = 136 if mode == "f16" else 132
    slab = np.full((X, ZROWS, 130), big, np.float32)
    zlo = z0 - RADIUS
    zs_lo, zs_hi = max(0, zlo), min(128, z0 + ZSLAB + RADIUS)
    # rows (z_local) x cols (y)
    slab[:, zs_lo - zlo : zs_hi - zlo, 2:130] = vol[:, :, zs_lo:zs_hi].transpose(0, 2, 1)
    xs = np.full((X, nv, ZROWS, wid), big, np_x)
    for dx in range(-RADIUS, RADIUS + 1):
        var = np.full((X, ZROWS, 130), big, np.float32)
        if dx >= 0:
            var[: X - dx] = slab[dx:]
        else:
            var[-dx:] = slab[: X + dx]
        if mode == "f16":
            v = (dx + RADIUS) * 2
            xs[:, v, :, 0:130] = var  # parity 0: y_real at col 2
            xs[:, v + 1, :, 1:131] = var  # parity 1: y_real at col 3
        else:
            xs[:, dx + RADIUS, :, 0:130] = var
    return xs.reshape(X, nv * ZROWS, wid)


def kernel(input_img, sigma_x, sigma_y, sigma_z, color_sigma):
    global LAST_RESULTS
    img = np.asarray(input_img, dtype=np.float32)
    B = img.shape[0]
    sx = float(np.asarray(sigma_x))
    sy = float(np.asarray(sigma_y))
    sz = float(np.asarray(sigma_z))
    cs = float(np.asarray(color_sigma))
    c = 1.0 / (2.0 * cs * cs)

    xmax = float(np.abs(img).max())
    big = xmax + np.sqrt(95.0 / c)

    if PAIRS and MODE == "f16":
        key = "pairs"
        if key not in _PROG_CACHE:
            _PROG_CACHE[key] = _build_program_pairs()
        nc = _PROG_CACHE[key]
        cbsv = np.full((X, 1), -c, np.float32)
        eye = np.eye(128, dtype=np.float32)
        widv = np.empty((X, NCLS, 128), np.float32)
        for i, (px, py, pz) in enumerate(_CLS_PATS):
            wsp = np.exp(-(px * px / (2 * sx * sx) + py * py / (2 * sy * sy)
                           + pz * pz / (2 * sz * sz)))
            widv[:, i, :] = wsp * eye
        widv = widv.astype(np.float16).reshape(X, NCLS * 128)
        in_maps = []
        for core in range(8):
            b, q = divmod(core, 4)
            xsv = _prep_core_inputs_pairs(img[b, 0], q * ZSLAB, big)
            in_maps.append({"xs": xsv, "cbs": cbsv, "wids": widv})
    else:
        if MODE not in _PROG_CACHE:
            _PROG_CACHE[MODE] = _build_program(MODE)
        nc, np_x = _PROG_CACHE[MODE]

        # per-tap log spatial weights and exp scale
        cbv = np.zeros((X, NTAPS + 1), np.float32)
        for k, (dx, dy, dz) in enumerate(_TAPS):
            cbv[:, k] = -(dx * dx / (2 * sx * sx) + dy * dy / (2 * sy * sy)
                          + dz * dz / (2 * sz * sz))
        cbv[:, NTAPS] = -c

        eye = np.eye(X, dtype=np.float32)
        in_maps = []
        for core in range(8):
            b, q = divmod(core, 4)
            xs = _prep_core_inputs(img[b, 0], q * ZSLAB, big, np_x, MODE)
            in_maps.append({"xs": xs, "cb": cbv, "ident": eye})

    res = bass_utils.run_bass_kernel_spmd(
        nc, in_maps, core_ids=list(range(8)), trace=TRACE
    )
    LAST_RESULTS = res

    outv = np.empty_like(img)
    for core in range(8):
        b, q = divmod(core, 4)
        o = res.results[core]["out"].reshape(X, ZSLAB, 128)  # (x, z_local, y)
        outv[b, 0, :, :, q * ZSLAB : (q + 1) * ZSLAB] = o.transpose(0, 2, 1)
    return outv


# revision 9
# speedup vs baseline: 2.7437x; 2.7437x over previous
"""3D bilateral filter (RADIUS=2, 5x5x5 window) on 8 Trainium2 NeuronCores.

Sharding: 8 cores = 2 batches x 4 z-slabs of 32 (halo 2 handled host-side).
Per-core layout: partitions = x (128), free dim = (z_local rows) x (padded y).
Out-of-volume taps are neutralized by padding with a large value BIG chosen so
the range weight exp(-c*(x-BIG)^2 + b) underflows to exactly 0 on the ACT LUT.
x-axis tap shifts are pre-materialized host-side as 5 shifted variants (plus a
second y-parity copy in fp16 mode, keeping DVE reads 4B-aligned for 2x mode).

Per tap on-chip:  D = x - x_shift (DVE), S = D^2 (ACT Square), W = exp(-c*S+b)
(ACT Exp, b = log spatial weight), P = W * x_shift (DVE), then num += P and
den += W via identity-matmul accumulation into PSUM (PE does all adds).
Finally out = num * reciprocal(den) (DVE) and DMA out.
"""

import os
import sys

import numpy as np

for _p in ("/root/.axon_site", "/root/.axon_site/_ro/trn_rl_repo",
           "/root/.axon_site/_ro/pypackages", "/opt/trn_rl_repo"):
    if os.path.isdir(_p) and _p not in sys.path:
        sys.path.append(_p)

import concourse.bacc as bacc
import concourse.mybir as mybir
from concourse.tile import TileContext
from concourse import bass_utils

RADIUS = 2
NTAPS = 5 * 5 * 5
X = 128  # partitions (dim 2 of input)
ZSLAB = 32  # output z rows per core
ZROWS = ZSLAB + 2 * RADIUS  # z rows incl halo
BLK = 16  # z rows per PSUM block
NBLK = ZSLAB // BLK

MODE = os.environ.get("BILAT_MODE", "f16")  # "f16" or "f32"
PAIRS = bool(int(os.environ.get("BILAT_PAIRS", "1")))  # pair-sharing kernel
TRACE = bool(int(os.environ.get("BILAT_TRACE", "0")))

LAST_RESULTS = None  # BassKernelResults of most recent run (for test.py)

_TAPS = [(dx, dy, dz)
         for dx in range(-RADIUS, RADIUS + 1)
         for dy in range(-RADIUS, RADIUS + 1)
         for dz in range(-RADIUS, RADIUS + 1)]

# canonical pair representatives: o lexicographically positive (dx in {0,1,2})
_PAIRS_O = [o for o in _TAPS if o > (0, 0, 0)]
_CLS_PATS = [(0, 0, 0)] + sorted({(abs(a), abs(b), abs(c)) for a, b, c in _PAIRS_O})
_CLS_IDX = {p: i for i, p in enumerate(_CLS_PATS)}
NCLS = len(_CLS_PATS)

_PROG_CACHE = {}


def _build_program(mode):
    f32 = mybir.dt.float32
    f32r = mybir.dt.float32r
    f16 = mybir.dt.float16
    if mode == "f16":
        dt_x, dt_wp, dt_id, nv, wid = f16, f16, f16, 10, 136
    else:
        dt_x, dt_wp, dt_id, nv, wid = f32, f32r, f32r, 5, 132
    np_x = mybir.dt.np(dt_x)

    nc = bacc.Bacc("TRN2", target_bir_lowering=False, debug=False, num_devices=8)
    xs = nc.dram_tensor("xs", [X, nv * ZROWS, wid], dt_x, kind="ExternalInput")
    cb = nc.dram_tensor("cb", [X, NTAPS + 1], f32, kind="ExternalInput")
    ident = nc.dram_tensor("ident", [X, X], f32, kind="ExternalInput")
    out = nc.dram_tensor("out", [X, ZSLAB * 128], f32, kind="ExternalOutput")

    Sq = mybir.ActivationFunctionType.Square
    Ex = mybir.ActivationFunctionType.Exp

    nb = 3 if mode == "f16" else 2
    with TileContext(nc) as tc:
        with (
            tc.tile_pool(name="big", bufs=1) as bigpool,
            tc.tile_pool(name="dd", bufs=nb) as dpool,
            tc.tile_pool(name="ss", bufs=nb) as spool,
            tc.tile_pool(name="ww", bufs=nb) as wpool,
            tc.tile_pool(name="pp", bufs=nb) as ppool,
            tc.tile_pool(name="ev", bufs=1) as epool,
            tc.tile_pool(name="ps", bufs=1, space="PSUM") as psp,
        ):
            xs_t = bigpool.tile([X, nv * ZROWS, wid], dt_x)
            nc.sync.dma_start(out=xs_t, in_=xs.ap())
            cb_t = bigpool.tile([X, NTAPS + 1], f32)
            nc.sync.dma_start(out=cb_t, in_=cb.ap())
            id_f32 = bigpool.tile([X, X], f32)
            nc.sync.dma_start(out=id_f32, in_=ident.ap())
            id_t = bigpool.tile([X, X], dt_id)
            nc.vector.tensor_copy(out=id_t, in_=id_f32)

            def read_ap(dx, dy, dz, blk):
                # AP into xs_t for tap (dx,dy,dz), z-block blk: [128,BLK,128]
                if mode == "f16":
                    v = (dx + RADIUS) * 2 + (dy & 1)
                    col0 = 2 + dy + (dy & 1)
                else:
                    v = dx + RADIUS
                    col0 = 2 + dy
                r0 = v * ZROWS + RADIUS + dz + BLK * blk
                return xs_t[:, r0 : r0 + BLK, col0 : col0 + 128]

            for blk in range(NBLK):
                p_num = psp.tile([X, BLK, 128], mybir.dt.float32, tag="num")
                p_den = psp.tile([X, BLK, 128], mybir.dt.float32, tag="den")
                for k, (dx, dy, dz) in enumerate(_TAPS):
                    first = k == 0
                    last = k == NTAPS - 1
                    base = read_ap(0, 0, 0, blk)
                    shft = read_ap(dx, dy, dz, blk)
                    d_t = dpool.tile([X, BLK, 128], dt_x)
                    nc.vector.tensor_sub(out=d_t, in0=base, in1=shft)
                    # Balance the square op between DVE (fp16 2x) and ACT
                    sq_on_dve = mode == "f16" and (k % 12) < 5
                    if sq_on_dve:
                        s_t = spool.tile([X, BLK, 128], dt_x, tag="s16")
                        nc.vector.tensor_mul(out=s_t, in0=d_t, in1=d_t)
                    else:
                        s_t = spool.tile([X, BLK, 128], mybir.dt.float32, tag="s32")
                        nc.scalar.activation(s_t, d_t, Sq)
                    w_t = wpool.tile([X, BLK, 128], dt_wp)
                    nc.scalar.activation(
                        w_t, s_t, Ex,
                        bias=cb_t[:, k : k + 1],
                        scale=cb_t[:, NTAPS : NTAPS + 1],
                    )
                    p_t = ppool.tile([X, BLK, 128], dt_wp)
                    nc.vector.tensor_mul(out=p_t, in0=w_t, in1=shft)
                    for r in range(BLK // 4):
                        nc.tensor.matmul(
                            p_num[:, 4 * r : 4 * r + 4, :], id_t,
                            p_t[:, 4 * r : 4 * r + 4, :],
                            start=first, stop=last,
                        )
                        nc.tensor.matmul(
                            p_den[:, 4 * r : 4 * r + 4, :], id_t,
                            w_t[:, 4 * r : 4 * r + 4, :],
                            start=first, stop=last,
                        )
                rec_t = epool.tile([X, BLK, 128], mybir.dt.float32, tag="rec")
                nc.vector.reciprocal(out=rec_t, in_=p_den)
                o_t = epool.tile([X, BLK, 128], mybir.dt.float32, tag="out")
                nc.vector.tensor_mul(out=o_t, in0=p_num, in1=rec_t)
                nc.sync.dma_start(
                    out=out.ap()[:, BLK * 128 * blk : BLK * 128 * (blk + 1)],
                    in_=o_t,
                )
    nc.compile()
    return nc, np_x


def _build_program_pairs():
    """fp16 pair kernel: one sub/square/exp per +-tap pair. The reverse tap's
    weight field is obtained by DMA-shifting the exp output across partitions
    (x) and free dims (z,y); spatial weights ride in scaled-identity lhsT."""
    f32 = mybir.dt.float32
    f16 = mybir.dt.float16
    WID = 136  # y at col 4, pads 4+4
    EC = 132  # ext-region cols (y' in [-2,130))
    ER = BLK + 4  # ext-region rows
    PZ = ZROWS + 4  # variant rows: z' = row - 4, rows 0,1,38,39 always pad

    nc = bacc.Bacc("TRN2", target_bir_lowering=False, debug=False, num_devices=8)
    xs = nc.dram_tensor("xs", [X, 6 * PZ, WID], f16, kind="ExternalInput")
    cbs = nc.dram_tensor("cbs", [X, 1], f32, kind="ExternalInput")
    wids = nc.dram_tensor("wids", [X, NCLS * 128], f16, kind="ExternalInput")
    out = nc.dram_tensor("out", [X, ZSLAB * 128], f32, kind="ExternalOutput")

    Sq = mybir.ActivationFunctionType.Square
    Ex = mybir.ActivationFunctionType.Exp

    with TileContext(nc) as tc:
        with (
            tc.tile_pool(name="big", bufs=1) as bigpool,
            tc.tile_pool(name="de", bufs=4) as depool,
            tc.tile_pool(name="p1", bufs=3) as p1pool,
            tc.tile_pool(name="es", bufs=4) as espool,
            tc.tile_pool(name="p2", bufs=3) as p2pool,
            tc.tile_pool(name="ev", bufs=1) as epool,
            tc.tile_pool(name="ps", bufs=1, space="PSUM") as psp,
        ):
            xs_t = bigpool.tile([X, 6 * PZ, WID], f16)
            nc.sync.dma_start(out=xs_t, in_=xs.ap())
            cbs_t = bigpool.tile([X, 1], f32)
            nc.sync.dma_start(out=cbs_t, in_=cbs.ap())
            wid_t = bigpool.tile([X, NCLS * 128], f16)
            nc.sync.dma_start(out=wid_t, in_=wids.ap())
            ones_t = bigpool.tile([X, BLK, 128], f16)
            nc.gpsimd.memset(ones_t, 1.0)

            def rd(v, r0, nr, c0, ncol):
                return xs_t[:, v * PZ + r0 : v * PZ + r0 + nr, c0 : c0 + ncol]

            def lhs(pat):
                c = _CLS_IDX[pat]
                return wid_t[:, c * 128 : (c + 1) * 128]

            for blk in range(NBLK):
                R0 = blk * BLK + 2  # ext rows = xs rows [R0, R0+ER)
                p_num = psp.tile([X, BLK, 128], f32, tag="num")
                p_den = psp.tile([X, BLK, 128], f32, tag="den")
                # center tap: W = 1
                for r in range(BLK // 4):
                    nc.tensor.matmul(
                        p_num[:, 4 * r : 4 * r + 4, :], lhs((0, 0, 0)),
                        rd(0, R0 + 2 + 4 * r, 4, 4, 128),
                        start=True, stop=False,
                    )
                    nc.tensor.matmul(
                        p_den[:, 4 * r : 4 * r + 4, :], lhs((0, 0, 0)),
                        ones_t[:, 4 * r : 4 * r + 4, :],
                        start=True, stop=False,
                    )
                for pi, (dx, dy, dz) in enumerate(_PAIRS_O):
                    last = pi == len(_PAIRS_O) - 1
                    v, vn = 3 + dx, 3 - dx
                    cls = lhs((dx, abs(dy), abs(dz)))
                    # D on ext region [ER x EC], stored flat with 4-elem guards
                    # so the (dz,dy) shift below is one contiguous run.
                    d_t = depool.tile([X, 8 + ER * EC], f16)
                    dv = d_t[:, 4 : 4 + ER * EC].rearrange(
                        "p (r c) -> p r c", c=EC
                    )
                    nc.vector.tensor_sub(
                        out=dv,
                        in0=rd(0, R0, ER, 2, EC),
                        in1=rd(v, R0 + dz, ER, 2 + dy, EC),
                    )
                    # S = D^2 in place (split between DVE and ACT), E = exp(-c*S)
                    flat = d_t[:, 4 : 4 + ER * EC]
                    if pi % 7 < 2:
                        nc.vector.tensor_mul(out=flat, in0=flat, in1=flat)
                    else:
                        nc.scalar.activation(flat, flat, Sq)
                    nc.scalar.activation(flat, flat, Ex, scale=cbs_t[:, 0:1])
                    # P1 = E * A_shift on base region
                    p1_t = p1pool.tile([X, BLK, 128], f16)
                    nc.vector.tensor_mul(
                        out=p1_t,
                        in0=dv[:, 2 : 2 + BLK, 2:130],
                        in1=rd(v, R0 + 2 + dz, BLK, 4 + dy, 128),
                    )
                    # Es(j) = E(j - o): shifted copy = constant flat offset;
                    # row-wrap bleed lands in pad cols of es (never read).
                    es_t = espool.tile([X, BLK * EC], f16)
                    esv = es_t.rearrange("p (r c) -> p r c", c=EC)
                    off = 4 + (2 - dz) * EC - dy
                    if dx > 0:
                        nc.gpsimd.memset(es_t[0:dx], 0.0)
                    nc.sync.dma_start(
                        out=es_t[dx:64, :],
                        in_=d_t[0 : 64 - dx, off : off + BLK * EC],
                    )
                    nc.sync.dma_start(
                        out=es_t[64:X, :],
                        in_=d_t[64 - dx : X - dx, off : off + BLK * EC],
                    )
                    # P2s = Es * A(j-o)
                    p2_t = p2pool.tile([X, BLK, 128], f16)
                    nc.vector.tensor_mul(
                        out=p2_t,
                        in0=esv[:, :, 2:130],
                        in1=rd(vn, R0 + 2 - dz, BLK, 4 - dy, 128),
                    )
                    for r in range(BLK // 4):
                        sl = slice(4 * r, 4 * r + 4)
                        nc.tensor.matmul(
                            p_num[:, sl, :], cls, p1_t[:, sl, :],
                            start=False, stop=False,
                        )
                        nc.tensor.matmul(
                            p_num[:, sl, :], cls, p2_t[:, sl, :],
                            start=False, stop=last,
                        )
                        nc.tensor.matmul(
                            p_den[:, sl, :], cls,
                            dv[:, 2 + 4 * r : 6 + 4 * r, 2:130],
                            start=False, stop=False,
                        )
                        nc.tensor.matmul(
                            p_den[:, sl, :], cls,
                            esv[:, sl, 2:130],
                            start=False, stop=last,
                        )
                rec_t = epool.tile([X, BLK, 128], f32, tag="rec")
                nc.vector.reciprocal(out=rec_t, in_=p_den)
                o_t = epool.tile([X, BLK, 128], f32, tag="out")
                nc.vector.tensor_mul(out=o_t, in0=p_num, in1=rec_t)
                nc.sync.dma_start(
                    out=out.ap()[:, BLK * 128 * blk : BLK * 128 * (blk + 1)],
                    in_=o_t,
                )
    nc.compile()
    return nc


def _prep_core_inputs_pairs(vol, z0, big):
    """Variants for the pair kernel: index 0 = base (+BIG pads), 1..5 = x-shift
    dx=-2..2 (-BIG pads). Width 136, y_real at col 4, z_local at row 2."""
    WID = 136
    PZ = ZROWS + 4
    data = np.empty((X, PZ, 128), np.float32)
    valid = np.zeros((PZ,), bool)
    zlo = z0 - 4  # row r holds z' = r - 4
    zs_lo, zs_hi = max(0, z0 - RADIUS), min(128, z0 + ZSLAB + RADIUS)
    data[:, zs_lo - zlo : zs_hi - zlo] = vol[:, :, zs_lo:zs_hi].transpose(0, 2, 1)
    valid[zs_lo - zlo : zs_hi - zlo] = True

    xs = np.empty((X, 6, PZ, WID), np.float32)
    xs[:, 0] = big
    xs[:, 0, valid, 4:132] = data[:, valid]
    for dx in range(-RADIUS, RADIUS + 1):
        vi = 3 + dx
        xs[:, vi] = -big
        if dx >= 0:
            xs[: X - dx, vi, valid, 4:132] = data[dx:][:, valid]
        else:
            xs[-dx:, vi, valid, 4:132] = data[: X + dx][:, valid]
    return xs.astype(np.float16).reshape(X, 6 * PZ, WID)


def _prep_core_inputs(vol, z0, big, np_x, mode):
    """vol: (128,128,128) f32 volume (x,y,z) for one batch. Returns xs array."""
    nv = 10 if mode == "f16" else 5
    wid = 136 if mode == "f16" else 132
    slab = np.full((X, ZROWS, 130), big, np.float32)
    zlo = z0 - RADIUS
    zs_lo, zs_hi = max(0, zlo), min(128, z0 + ZSLAB + RADIUS)
    # rows (z_local) x cols (y)
    slab[:, zs_lo - zlo : zs_hi - zlo, 2:130] = vol[:, :, zs_lo:zs_hi].transpose(0, 2, 1)
    xs = np.full((X, nv, ZROWS, wid), big, np_x)
    for dx in range(-RADIUS, RADIUS + 1):
        var = np.full((X, ZROWS, 130), big, np.float32)
        if dx >= 0:
            var[: X - dx] = slab[dx:]
        else:
            var[-dx:] = slab[: X + dx]
        if mode == "f16":
            v = (dx + RADIUS) * 2
            xs[:, v, :, 0:130] = var  # parity 0: y_real at col 2
            xs[:, v + 1, :, 1:131] = var  # parity 1: y_real at col 3
        else:
            xs[:, dx + RADIUS, :, 0:130] = var
    return xs.reshape(X, nv * ZROWS, wid)


def kernel(input_img, sigma_x, sigma_y, sigma_z, color_sigma):
    global LAST_RESULTS
    img = np.asarray(input_img, dtype=np.float32)
    B = img.shape[0]
    sx = float(np.asarray(sigma_x))
    sy = float(np.asarray(sigma_y))
    sz = float(np.asarray(sigma_z))
    cs = float(np.asarray(color_sigma))
    c = 1.0 / (2.0 * cs * cs)

    xmax = float(np.abs(img).max())
    big = xmax + np.sqrt(95.0 / c)

    if PAIRS and MODE == "f16":
        key = "pairs"
        if key not in _PROG_CACHE:
            _PROG_CACHE[key] = _build_program_pairs()
        nc = _PROG_CACHE[key]
        cbsv = np.full((X, 1), -c, np.float32)
        eye = np.eye(128, dtype=np.float32)
        widv = np.empty((X, NCLS, 128), np.float32)
        for i, (px, py, pz) in enumerate(_CLS_PATS):
            wsp = np.exp(-(px * px / (2 * sx * sx) + py * py / (2 * sy * sy)
                           + pz * pz / (2 * sz * sz)))
            widv[:, i, :] = wsp * eye
        widv = widv.astype(np.float16).reshape(X, NCLS * 128)
        in_maps = []
        for core in range(8):
            b, q = divmod(core, 4)
            xsv = _prep_core_inputs_pairs(img[b, 0], q * ZSLAB, big)
            in_maps.append({"xs": xsv, "cbs": cbsv, "wids": widv})
    else:
        if MODE not in _PROG_CACHE:
            _PROG_CACHE[MODE] = _build_program(MODE)
        nc, np_x = _PROG_CACHE[MODE]

        # per-tap log spatial weights and exp scale
        cbv = np.zeros((X, NTAPS + 1), np.float32)
        for k, (dx, dy, dz) in enumerate(_TAPS):
            cbv[:, k] = -(dx * dx / (2 * sx * sx) + dy * dy / (2 * sy * sy)
                          + dz * dz / (2 * sz * sz))
        cbv[:, NTAPS] = -c

        eye = np.eye(X, dtype=np.float32)
        in_maps = []
        for core in range(8):
            b, q = divmod(core, 4)
            xs = _prep_core_inputs(img[b, 0], q * ZSLAB, big, np_x, MODE)
            in_maps.append({"xs": xs, "cb": cbv, "ident": eye})

    res = bass_utils.run_bass_kernel_spmd(
        nc, in_maps, core_ids=list(range(8)), trace=TRACE
    )
    LAST_RESULTS = res

    outv = np.empty_like(img)
    for core in range(8):
        b, q = divmod(core, 4)
        o = res.results[core]["out"].reshape(X, ZSLAB, 128)  # (x, z_local, y)
        outv[b, 0, :, :, q * ZSLAB : (q + 1) * ZSLAB] = o.transpose(0, 2, 1)
    return outv


# revision 12
# speedup vs baseline: 2.8437x; 1.0364x over previous
"""3D bilateral filter (RADIUS=2, 5x5x5 window) on 8 Trainium2 NeuronCores.

Sharding: 8 cores = 2 batches x 4 z-slabs of 32 (halo 2 handled host-side).
Per-core layout: partitions = x (128), free dim = (z_local rows) x (padded y).
Out-of-volume taps are neutralized by padding with a large value BIG chosen so
the range weight exp(-c*(x-BIG)^2 + b) underflows to exactly 0 on the ACT LUT.
x-axis tap shifts are pre-materialized host-side as 5 shifted variants (plus a
second y-parity copy in fp16 mode, keeping DVE reads 4B-aligned for 2x mode).

Per tap on-chip:  D = x - x_shift (DVE), S = D^2 (ACT Square), W = exp(-c*S+b)
(ACT Exp, b = log spatial weight), P = W * x_shift (DVE), then num += P and
den += W via identity-matmul accumulation into PSUM (PE does all adds).
Finally out = num * reciprocal(den) (DVE) and DMA out.
"""

import os
import sys

import numpy as np

for _p in ("/root/.axon_site", "/root/.axon_site/_ro/trn_rl_repo",
           "/root/.axon_site/_ro/pypackages", "/opt/trn_rl_repo"):
    if os.path.isdir(_p) and _p not in sys.path:
        sys.path.append(_p)

import concourse.bacc as bacc
import concourse.mybir as mybir
from concourse.tile import TileContext
from concourse import bass_utils

RADIUS = 2
NTAPS = 5 * 5 * 5
X = 128  # partitions (dim 2 of input)
ZSLAB = 32  # output z rows per core
ZROWS = ZSLAB + 2 * RADIUS  # z rows incl halo
BLK = 16  # z rows per PSUM block
NBLK = ZSLAB // BLK

MODE = os.environ.get("BILAT_MODE", "f16")  # "f16" or "f32"
PAIRS = bool(int(os.environ.get("BILAT_PAIRS", "1")))  # pair-sharing kernel
TRACE = bool(int(os.environ.get("BILAT_TRACE", "0")))

LAST_RESULTS = None  # BassKernelResults of most recent run (for test.py)

_TAPS = [(dx, dy, dz)
         for dx in range(-RADIUS, RADIUS + 1)
         for dy in range(-RADIUS, RADIUS + 1)
         for dz in range(-RADIUS, RADIUS + 1)]

# canonical pair representatives: o lexicographically positive (dx in {0,1,2})
_PAIRS_O = [o for o in _TAPS if o > (0, 0, 0)]
_CLS_PATS = [(0, 0, 0)] + sorted({(abs(a), abs(b), abs(c)) for a, b, c in _PAIRS_O})
_CLS_IDX = {p: i for i, p in enumerate(_CLS_PATS)}
NCLS = len(_CLS_PATS)

_PROG_CACHE = {}


def _build_program(mode):
    f32 = mybir.dt.float32
    f32r = mybir.dt.float32r
    f16 = mybir.dt.float16
    if mode == "f16":
        dt_x, dt_wp, dt_id, nv, wid = f16, f16, f16, 10, 136
    else:
        dt_x, dt_wp, dt_id, nv, wid = f32, f32r, f32r, 5, 132
    np_x = mybir.dt.np(dt_x)

    nc = bacc.Bacc("TRN2", target_bir_lowering=False, debug=False, num_devices=8)
    xs = nc.dram_tensor("xs", [X, nv * ZROWS, wid], dt_x, kind="ExternalInput")
    cb = nc.dram_tensor("cb", [X, NTAPS + 1], f32, kind="ExternalInput")
    ident = nc.dram_tensor("ident", [X, X], f32, kind="ExternalInput")
    out = nc.dram_tensor("out", [X, ZSLAB * 128], f32, kind="ExternalOutput")

    Sq = mybir.ActivationFunctionType.Square
    Ex = mybir.ActivationFunctionType.Exp

    nb = 3 if mode == "f16" else 2
    with TileContext(nc) as tc:
        with (
            tc.tile_pool(name="big", bufs=1) as bigpool,
            tc.tile_pool(name="dd", bufs=nb) as dpool,
            tc.tile_pool(name="ss", bufs=nb) as spool,
            tc.tile_pool(name="ww", bufs=nb) as wpool,
            tc.tile_pool(name="pp", bufs=nb) as ppool,
            tc.tile_pool(name="ev", bufs=1) as epool,
            tc.tile_pool(name="ps", bufs=1, space="PSUM") as psp,
        ):
            xs_t = bigpool.tile([X, nv * ZROWS, wid], dt_x)
            nc.sync.dma_start(out=xs_t, in_=xs.ap())
            cb_t = bigpool.tile([X, NTAPS + 1], f32)
            nc.sync.dma_start(out=cb_t, in_=cb.ap())
            id_f32 = bigpool.tile([X, X], f32)
            nc.sync.dma_start(out=id_f32, in_=ident.ap())
            id_t = bigpool.tile([X, X], dt_id)
            nc.vector.tensor_copy(out=id_t, in_=id_f32)

            def read_ap(dx, dy, dz, blk):
                # AP into xs_t for tap (dx,dy,dz), z-block blk: [128,BLK,128]
                if mode == "f16":
                    v = (dx + RADIUS) * 2 + (dy & 1)
                    col0 = 2 + dy + (dy & 1)
                else:
                    v = dx + RADIUS
                    col0 = 2 + dy
                r0 = v * ZROWS + RADIUS + dz + BLK * blk
                return xs_t[:, r0 : r0 + BLK, col0 : col0 + 128]

            for blk in range(NBLK):
                p_num = psp.tile([X, BLK, 128], mybir.dt.float32, tag="num")
                p_den = psp.tile([X, BLK, 128], mybir.dt.float32, tag="den")
                for k, (dx, dy, dz) in enumerate(_TAPS):
                    first = k == 0
                    last = k == NTAPS - 1
                    base = read_ap(0, 0, 0, blk)
                    shft = read_ap(dx, dy, dz, blk)
                    d_t = dpool.tile([X, BLK, 128], dt_x)
                    nc.vector.tensor_sub(out=d_t, in0=base, in1=shft)
                    # Balance the square op between DVE (fp16 2x) and ACT
                    sq_on_dve = mode == "f16" and (k % 12) < 5
                    if sq_on_dve:
                        s_t = spool.tile([X, BLK, 128], dt_x, tag="s16")
                        nc.vector.tensor_mul(out=s_t, in0=d_t, in1=d_t)
                    else:
                        s_t = spool.tile([X, BLK, 128], mybir.dt.float32, tag="s32")
                        nc.scalar.activation(s_t, d_t, Sq)
                    w_t = wpool.tile([X, BLK, 128], dt_wp)
                    nc.scalar.activation(
                        w_t, s_t, Ex,
                        bias=cb_t[:, k : k + 1],
                        scale=cb_t[:, NTAPS : NTAPS + 1],
                    )
                    p_t = ppool.tile([X, BLK, 128], dt_wp)
                    nc.vector.tensor_mul(out=p_t, in0=w_t, in1=shft)
                    for r in range(BLK // 4):
                        nc.tensor.matmul(
                            p_num[:, 4 * r : 4 * r + 4, :], id_t,
                            p_t[:, 4 * r : 4 * r + 4, :],
                            start=first, stop=last,
                        )
                        nc.tensor.matmul(
                            p_den[:, 4 * r : 4 * r + 4, :], id_t,
                            w_t[:, 4 * r : 4 * r + 4, :],
                            start=first, stop=last,
                        )
                rec_t = epool.tile([X, BLK, 128], mybir.dt.float32, tag="rec")
                nc.vector.reciprocal(out=rec_t, in_=p_den)
                o_t = epool.tile([X, BLK, 128], mybir.dt.float32, tag="out")
                nc.vector.tensor_mul(out=o_t, in0=p_num, in1=rec_t)
                nc.sync.dma_start(
                    out=out.ap()[:, BLK * 128 * blk : BLK * 128 * (blk + 1)],
                    in_=o_t,
                )
    nc.compile()
    return nc, np_x


def _build_program_pairs():
    """fp16 pair kernel: one sub/square/exp per +-tap pair. The reverse tap's
    weight field is obtained by DMA-shifting the exp output across partitions
    (x) and free dims (z,y); spatial weights ride in scaled-identity lhsT."""
    f32 = mybir.dt.float32
    f16 = mybir.dt.float16
    WID = 136  # y at col 4, pads 4+4
    EC = 132  # ext-region cols (y' in [-2,130))
    ER = BLK + 4  # ext-region rows
    PZ = ZROWS + 4  # variant rows: z' = row - 4, rows 0,1,38,39 always pad

    nc = bacc.Bacc("TRN2", target_bir_lowering=False, debug=False, num_devices=8)
    xs = nc.dram_tensor("xs", [X, 6 * PZ, WID], f16, kind="ExternalInput")
    cbs = nc.dram_tensor("cbs", [X, 1], f32, kind="ExternalInput")
    wids = nc.dram_tensor("wids", [X, NCLS * 128], f16, kind="ExternalInput")
    out = nc.dram_tensor("out", [X, ZSLAB * 128], f32, kind="ExternalOutput")

    Sq = mybir.ActivationFunctionType.Square
    Ex = mybir.ActivationFunctionType.Exp

    with TileContext(nc) as tc:
        with (
            tc.tile_pool(name="big", bufs=1) as bigpool,
            tc.tile_pool(name="de", bufs=5) as depool,
            tc.tile_pool(name="p1", bufs=3) as p1pool,
            tc.tile_pool(name="es", bufs=4) as espool,
            tc.tile_pool(name="p2", bufs=3) as p2pool,
            tc.tile_pool(name="ev", bufs=1) as epool,
            tc.tile_pool(name="ps", bufs=1, space="PSUM") as psp,
        ):
            xs_t = bigpool.tile([X, 6 * PZ, WID], f16)
            nc.sync.dma_start(out=xs_t, in_=xs.ap())
            cbs_t = bigpool.tile([X, 1], f32)
            nc.sync.dma_start(out=cbs_t, in_=cbs.ap())
            wid_t = bigpool.tile([X, NCLS * 128], f16)
            nc.sync.dma_start(out=wid_t, in_=wids.ap())
            ones_t = bigpool.tile([X, BLK, 128], f16)
            nc.gpsimd.memset(ones_t, 1.0)

            def rd(v, r0, nr, c0, ncol):
                return xs_t[:, v * PZ + r0 : v * PZ + r0 + nr, c0 : c0 + ncol]

            def lhs(pat):
                c = _CLS_IDX[pat]
                return wid_t[:, c * 128 : (c + 1) * 128]

            for blk in range(NBLK):
                R0 = blk * BLK + 2  # ext rows = xs rows [R0, R0+ER)
                p_num = psp.tile([X, BLK, 128], f32, tag="num")
                p_den = psp.tile([X, BLK, 128], f32, tag="den")
                # center tap: W = 1
                for r in range(BLK // 4):
                    nc.tensor.matmul(
                        p_num[:, 4 * r : 4 * r + 4, :], lhs((0, 0, 0)),
                        rd(0, R0 + 2 + 4 * r, 4, 4, 128),
                        start=True, stop=False,
                    )
                    nc.tensor.matmul(
                        p_den[:, 4 * r : 4 * r + 4, :], lhs((0, 0, 0)),
                        ones_t[:, 4 * r : 4 * r + 4, :],
                        start=True, stop=False,
                    )
                for pi, (dx, dy, dz) in enumerate(_PAIRS_O):
                    last = pi == len(_PAIRS_O) - 1
                    v, vn = 3 + dx, 3 - dx
                    cls = lhs((dx, abs(dy), abs(dz)))
                    # D on ext region [ER x EC], stored flat with 4-elem guards
                    # so the (dz,dy) shift below is one contiguous run.
                    d_t = depool.tile([X, 8 + ER * EC], f16)
                    dv = d_t[:, 4 : 4 + ER * EC].rearrange(
                        "p (r c) -> p r c", c=EC
                    )
                    nc.vector.tensor_sub(
                        out=dv,
                        in0=rd(0, R0, ER, 2, EC),
                        in1=rd(v, R0 + dz, ER, 2 + dy, EC),
                    )
                    # S = D^2 in place (split between DVE and ACT), E = exp(-c*S)
                    flat = d_t[:, 4 : 4 + ER * EC]
                    if pi % 7 < 2:
                        nc.vector.tensor_mul(out=flat, in0=flat, in1=flat)
                    else:
                        nc.scalar.activation(flat, flat, Sq)
                    nc.scalar.activation(flat, flat, Ex, scale=cbs_t[:, 0:1])
                    # P1 = E * A_shift on base region
                    p1_t = p1pool.tile([X, BLK, 128], f16)
                    nc.vector.tensor_mul(
                        out=p1_t,
                        in0=dv[:, 2 : 2 + BLK, 2:130],
                        in1=rd(v, R0 + 2 + dz, BLK, 4 + dy, 128),
                    )
                    # Es(j) = E(j - o). For dx=0 it is a pure free-dim shift:
                    # read E directly at offset APs. For dx>0, DMA-shift
                    # across partitions (constant flat offset; row-wrap bleed
                    # lands in pad cols, never read).
                    if dx == 0:
                        def es_sl(r0, nr):
                            return dv[:, 2 + r0 - dz : 2 + r0 - dz + nr,
                                      2 - dy : 130 - dy]
                    else:
                        es_t = espool.tile([X, BLK * EC], f16)
                        esv = es_t.rearrange("p (r c) -> p r c", c=EC)
                        off = 4 + (2 - dz) * EC - dy
                        nc.gpsimd.memset(es_t[0:dx], 0.0)
                        for a in range(0, X, 32):
                            lo = max(a, dx)
                            nc.sync.dma_start(
                                out=es_t[lo : a + 32, :],
                                in_=d_t[lo - dx : a + 32 - dx,
                                        off : off + BLK * EC],
                            )

                        def es_sl(r0, nr):
                            return esv[:, r0 : r0 + nr, 2:130]
                    # P2s = Es * A(j-o)
                    p2_t = p2pool.tile([X, BLK, 128], f16)
                    nc.vector.tensor_mul(
                        out=p2_t,
                        in0=es_sl(0, BLK),
                        in1=rd(vn, R0 + 2 - dz, BLK, 4 - dy, 128),
                    )
                    for r in range(BLK // 4):
                        sl = slice(4 * r, 4 * r + 4)
                        nc.tensor.matmul(
                            p_num[:, sl, :], cls, p1_t[:, sl, :],
                            start=False, stop=False,
                        )
                        nc.tensor.matmul(
                            p_num[:, sl, :], cls, p2_t[:, sl, :],
                            start=False, stop=last,
                        )
                        nc.tensor.matmul(
                            p_den[:, sl, :], cls,
                            dv[:, 2 + 4 * r : 6 + 4 * r, 2:130],
                            start=False, stop=False,
                        )
                        nc.tensor.matmul(
                            p_den[:, sl, :], cls,
                            es_sl(4 * r, 4),
                            start=False, stop=last,
                        )
                rec_t = epool.tile([X, BLK, 128], f32, tag="rec")
                nc.vector.reciprocal(out=rec_t, in_=p_den)
                o_t = epool.tile([X, BLK, 128], f32, tag="out")
                nc.vector.tensor_mul(out=o_t, in0=p_num, in1=rec_t)
                nc.sync.dma_start(
                    out=out.ap()[:, BLK * 128 * blk : BLK * 128 * (blk + 1)],
                    in_=o_t,
                )
    nc.compile()
    return nc


def _prep_core_inputs_pairs(vol, z0, big):
    """Variants for the pair kernel: index 0 = base (+BIG pads), 1..5 = x-shift
    dx=-2..2 (-BIG pads). Width 136, y_real at col 4, z_local at row 2."""
    WID = 136
    PZ = ZROWS + 4
    data = np.empty((X, PZ, 128), np.float32)
    valid = np.zeros((PZ,), bool)
    zlo = z0 - 4  # row r holds z' = r - 4
    zs_lo, zs_hi = max(0, z0 - RADIUS), min(128, z0 + ZSLAB + RADIUS)
    data[:, zs_lo - zlo : zs_hi - zlo] = vol[:, :, zs_lo:zs_hi].transpose(0, 2, 1)
    valid[zs_lo - zlo : zs_hi - zlo] = True

    xs = np.empty((X, 6, PZ, WID), np.float32)
    xs[:, 0] = big
    xs[:, 0, valid, 4:132] = data[:, valid]
    for dx in range(-RADIUS, RADIUS + 1):
        vi = 3 + dx
        xs[:, vi] = -big
        if dx >= 0:
            xs[: X - dx, vi, valid, 4:132] = data[dx:][:, valid]
        else:
            xs[-dx:, vi, valid, 4:132] = data[: X + dx][:, valid]
    return xs.astype(np.float16).reshape(X, 6 * PZ, WID)


def _prep_core_inputs(vol, z0, big, np_x, mode):
    """vol: (128,128,128) f32 volume (x,y,z) for one batch. Returns xs array."""
    nv = 10 if mode == "f16" else 5
    wid = 136 if mode == "f16" else 132
    slab = np.full((X, ZROWS, 130), big, np.float32)
    zlo = z0 - RADIUS
    zs_lo, zs_hi = max(0, zlo), min(128, z0 + ZSLAB + RADIUS)
    # rows (z_local) x cols (y)
    slab[:, zs_lo - zlo : zs_hi - zlo, 2:130] = vol[:, :, zs_lo:zs_hi].transpose(0, 2, 1)
    xs = np.full((X, nv, ZROWS, wid), big, np_x)
    for dx in range(-RADIUS, RADIUS + 1):
        var = np.full((X, ZROWS, 130), big, np.float32)
        if dx >= 0:
            var[: X - dx] = slab[dx:]
        else:
            var[-dx:] = slab[: X + dx]
        if mode == "f16":
            v = (dx + RADIUS) * 2
            xs[:, v, :, 0:130] = var  # parity 0: y_real at col 2
            xs[:, v + 1, :, 1:131] = var  # parity 1: y_real at col 3
        else:
            xs[:, dx + RADIUS, :, 0:130] = var
    return xs.reshape(X, nv * ZROWS, wid)


def kernel(input_img, sigma_x, sigma_y, sigma_z, color_sigma):
    global LAST_RESULTS
    img = np.asarray(input_img, dtype=np.float32)
    B = img.shape[0]
    sx = float(np.asarray(sigma_x))
    sy = float(np.asarray(sigma_y))
    sz = float(np.asarray(sigma_z))
    cs = float(np.asarray(color_sigma))
    c = 1.0 / (2.0 * cs * cs)

    xmax = float(np.abs(img).max())
    big = xmax + np.sqrt(95.0 / c)

    if PAIRS and MODE == "f16":
        key = "pairs"
        if key not in _PROG_CACHE:
            _PROG_CACHE[key] = _build_program_pairs()
        nc = _PROG_CACHE[key]
        cbsv = np.full((X, 1), -c, np.float32)
        eye = np.eye(128, dtype=np.float32)
        widv = np.empty((X, NCLS, 128), np.float32)
        for i, (px, py, pz) in enumerate(_CLS_PATS):
            wsp = np.exp(-(px * px / (2 * sx * sx) + py * py / (2 * sy * sy)
                           + pz * pz / (2 * sz * sz)))
            widv[:, i, :] = wsp * eye
        widv = widv.astype(np.float16).reshape(X, NCLS * 128)
        in_maps = []
        for core in range(8):
            b, q = divmod(core, 4)
            xsv = _prep_core_inputs_pairs(img[b, 0], q * ZSLAB, big)
            in_maps.append({"xs": xsv, "cbs": cbsv, "wids": widv})
    else:
        if MODE not in _PROG_CACHE:
            _PROG_CACHE[MODE] = _build_program(MODE)
        nc, np_x = _PROG_CACHE[MODE]

        # per-tap log spatial weights and exp scale
        cbv = np.zeros((X, NTAPS + 1), np.float32)
        for k, (dx, dy, dz) in enumerate(_TAPS):
            cbv[:, k] = -(dx * dx / (2 * sx * sx) + dy * dy / (2 * sy * sy)
                          + dz * dz / (2 * sz * sz))
        cbv[:, NTAPS] = -c

        eye = np.eye(X, dtype=np.float32)
        in_maps = []
        for core in range(8):
            b, q = divmod(core, 4)
            xs = _prep_core_inputs(img[b, 0], q * ZSLAB, big, np_x, MODE)
            in_maps.append({"xs": xs, "cb": cbv, "ident": eye})

    res = bass_utils.run_bass_kernel_spmd(
        nc, in_maps, core_ids=list(range(8)), trace=TRACE
    )
    LAST_RESULTS = res

    outv = np.empty_like(img)
    for core in range(8):
        b, q = divmod(core, 4)
        o = res.results[core]["out"].reshape(X, ZSLAB, 128)  # (x, z_local, y)
        outv[b, 0, :, :, q * ZSLAB : (q + 1) * ZSLAB] = o.transpose(0, 2, 1)
    return outv


# revision 17
# speedup vs baseline: 3.0113x; 1.0590x over previous
"""3D bilateral filter (RADIUS=2, 5x5x5 window) on 8 Trainium2 NeuronCores.

Sharding: 8 cores = 2 batches x 4 z-slabs of 32 (halo 2 handled host-side).
Per-core layout: partitions = x (128), free dim = (z_local rows) x (padded y).
Out-of-volume taps are neutralized by padding with a large value BIG chosen so
the range weight exp(-c*(x-BIG)^2 + b) underflows to exactly 0 on the ACT LUT.
x-axis tap shifts are pre-materialized host-side as 5 shifted variants (plus a
second y-parity copy in fp16 mode, keeping DVE reads 4B-aligned for 2x mode).

Per tap on-chip:  D = x - x_shift (DVE), S = D^2 (ACT Square), W = exp(-c*S+b)
(ACT Exp, b = log spatial weight), P = W * x_shift (DVE), then num += P and
den += W via identity-matmul accumulation into PSUM (PE does all adds).
Finally out = num * reciprocal(den) (DVE) and DMA out.
"""

import os
import sys

import numpy as np

for _p in ("/root/.axon_site", "/root/.axon_site/_ro/trn_rl_repo",
           "/root/.axon_site/_ro/pypackages", "/opt/trn_rl_repo"):
    if os.path.isdir(_p) and _p not in sys.path:
        sys.path.append(_p)

import concourse.bacc as bacc
import concourse.mybir as mybir
from concourse.tile import TileContext
from concourse import bass_utils

RADIUS = 2
NTAPS = 5 * 5 * 5
X = 128  # partitions (dim 2 of input)
ZSLAB = 32  # output z rows per core
ZROWS = ZSLAB + 2 * RADIUS  # z rows incl halo
BLK = 16  # z rows per PSUM block
NBLK = ZSLAB // BLK

MODE = os.environ.get("BILAT_MODE", "f16")  # "f16" or "f32"
PAIRS = bool(int(os.environ.get("BILAT_PAIRS", "1")))  # pair-sharing kernel
TRACE = bool(int(os.environ.get("BILAT_TRACE", "0")))

LAST_RESULTS = None  # BassKernelResults of most recent run (for test.py)

_TAPS = [(dx, dy, dz)
         for dx in range(-RADIUS, RADIUS + 1)
         for dy in range(-RADIUS, RADIUS + 1)
         for dz in range(-RADIUS, RADIUS + 1)]

# canonical pair representatives: o lexicographically positive (dx in {0,1,2})
_PAIRS_O = [o for o in _TAPS if o > (0, 0, 0)]
# interleave the 12 dx=0 pairs (no shift-DMA) among the 50 dx>0 pairs to
# smooth DMA ring load
_p0 = [o for o in _PAIRS_O if o[0] == 0]
_p1 = [o for o in _PAIRS_O if o[0] > 0]
_PAIRS_O = []
for _i in range(len(_p1)):
    _PAIRS_O.append(_p1[_i])
    if _i % 4 == 3 and _p0:
        _PAIRS_O.append(_p0.pop())
_PAIRS_O.extend(_p0)
del _p0, _p1
_CLS_PATS = [(0, 0, 0)] + sorted({(abs(a), abs(b), abs(c)) for a, b, c in _PAIRS_O})
_CLS_IDX = {p: i for i, p in enumerate(_CLS_PATS)}
NCLS = len(_CLS_PATS)

_PROG_CACHE = {}


def _build_program(mode):
    f32 = mybir.dt.float32
    f32r = mybir.dt.float32r
    f16 = mybir.dt.float16
    if mode == "f16":
        dt_x, dt_wp, dt_id, nv, wid = f16, f16, f16, 10, 136
    else:
        dt_x, dt_wp, dt_id, nv, wid = f32, f32r, f32r, 5, 132
    np_x = mybir.dt.np(dt_x)

    nc = bacc.Bacc("TRN2", target_bir_lowering=False, debug=False, num_devices=8)
    xs = nc.dram_tensor("xs", [X, nv * ZROWS, wid], dt_x, kind="ExternalInput")
    cb = nc.dram_tensor("cb", [X, NTAPS + 1], f32, kind="ExternalInput")
    ident = nc.dram_tensor("ident", [X, X], f32, kind="ExternalInput")
    out = nc.dram_tensor("out", [X, ZSLAB * 128], f32, kind="ExternalOutput")

    Sq = mybir.ActivationFunctionType.Square
    Ex = mybir.ActivationFunctionType.Exp

    nb = 3 if mode == "f16" else 2
    with TileContext(nc) as tc:
        with (
            tc.tile_pool(name="big", bufs=1) as bigpool,
            tc.tile_pool(name="dd", bufs=nb) as dpool,
            tc.tile_pool(name="ss", bufs=nb) as spool,
            tc.tile_pool(name="ww", bufs=nb) as wpool,
            tc.tile_pool(name="pp", bufs=nb) as ppool,
            tc.tile_pool(name="ev", bufs=1) as epool,
            tc.tile_pool(name="ps", bufs=1, space="PSUM") as psp,
        ):
            xs_t = bigpool.tile([X, nv * ZROWS, wid], dt_x)
            nc.sync.dma_start(out=xs_t, in_=xs.ap())
            cb_t = bigpool.tile([X, NTAPS + 1], f32)
            nc.sync.dma_start(out=cb_t, in_=cb.ap())
            id_f32 = bigpool.tile([X, X], f32)
            nc.sync.dma_start(out=id_f32, in_=ident.ap())
            id_t = bigpool.tile([X, X], dt_id)
            nc.vector.tensor_copy(out=id_t, in_=id_f32)

            def read_ap(dx, dy, dz, blk):
                # AP into xs_t for tap (dx,dy,dz), z-block blk: [128,BLK,128]
                if mode == "f16":
                    v = (dx + RADIUS) * 2 + (dy & 1)
                    col0 = 2 + dy + (dy & 1)
                else:
                    v = dx + RADIUS
                    col0 = 2 + dy
                r0 = v * ZROWS + RADIUS + dz + BLK * blk
                return xs_t[:, r0 : r0 + BLK, col0 : col0 + 128]

            for blk in range(NBLK):
                p_num = psp.tile([X, BLK, 128], mybir.dt.float32, tag="num")
                p_den = psp.tile([X, BLK, 128], mybir.dt.float32, tag="den")
                for k, (dx, dy, dz) in enumerate(_TAPS):
                    first = k == 0
                    last = k == NTAPS - 1
                    base = read_ap(0, 0, 0, blk)
                    shft = read_ap(dx, dy, dz, blk)
                    d_t = dpool.tile([X, BLK, 128], dt_x)
                    nc.vector.tensor_sub(out=d_t, in0=base, in1=shft)
                    # Balance the square op between DVE (fp16 2x) and ACT
                    sq_on_dve = mode == "f16" and (k % 12) < 5
                    if sq_on_dve:
                        s_t = spool.tile([X, BLK, 128], dt_x, tag="s16")
                        nc.vector.tensor_mul(out=s_t, in0=d_t, in1=d_t)
                    else:
                        s_t = spool.tile([X, BLK, 128], mybir.dt.float32, tag="s32")
                        nc.scalar.activation(s_t, d_t, Sq)
                    w_t = wpool.tile([X, BLK, 128], dt_wp)
                    nc.scalar.activation(
                        w_t, s_t, Ex,
                        bias=cb_t[:, k : k + 1],
                        scale=cb_t[:, NTAPS : NTAPS + 1],
                    )
                    p_t = ppool.tile([X, BLK, 128], dt_wp)
                    nc.vector.tensor_mul(out=p_t, in0=w_t, in1=shft)
                    for r in range(BLK // 4):
                        nc.tensor.matmul(
                            p_num[:, 4 * r : 4 * r + 4, :], id_t,
                            p_t[:, 4 * r : 4 * r + 4, :],
                            start=first, stop=last,
                        )
                        nc.tensor.matmul(
                            p_den[:, 4 * r : 4 * r + 4, :], id_t,
                            w_t[:, 4 * r : 4 * r + 4, :],
                            start=first, stop=last,
                        )
                rec_t = epool.tile([X, BLK, 128], mybir.dt.float32, tag="rec")
                nc.vector.reciprocal(out=rec_t, in_=p_den)
                o_t = epool.tile([X, BLK, 128], mybir.dt.float32, tag="out")
                nc.vector.tensor_mul(out=o_t, in0=p_num, in1=rec_t)
                nc.sync.dma_start(
                    out=out.ap()[:, BLK * 128 * blk : BLK * 128 * (blk + 1)],
                    in_=o_t,
                )
    nc.compile()
    return nc, np_x


def _build_program_pairs():
    """fp16 pair kernel: one sub/square/exp per +-tap pair. The reverse tap's
    weight field is obtained by DMA-shifting the exp output across partitions
    (x) and free dims (z,y); spatial weights ride in scaled-identity lhsT."""
    f32 = mybir.dt.float32
    f16 = mybir.dt.float16
    WID = 136  # y at col 4, pads 4+4
    EC = 132  # ext-region cols (y' in [-2,130))
    ER = BLK + 4  # ext-region rows
    PZ = ZROWS + 4  # variant rows: z' = row - 4, rows 0,1,38,39 always pad

    nc = bacc.Bacc("TRN2", target_bir_lowering=False, debug=False, num_devices=8)
    xs = nc.dram_tensor("xs", [X, 6 * PZ, WID], f16, kind="ExternalInput")
    cbs = nc.dram_tensor("cbs", [X, 1], f32, kind="ExternalInput")
    wids = nc.dram_tensor("wids", [X, NCLS * 128], f16, kind="ExternalInput")
    out = nc.dram_tensor("out", [X, ZSLAB * 128], f32, kind="ExternalOutput")

    Sq = mybir.ActivationFunctionType.Square
    Ex = mybir.ActivationFunctionType.Exp

    with TileContext(nc) as tc:
        with (
            tc.tile_pool(name="big", bufs=1) as bigpool,
            tc.tile_pool(name="de", bufs=6) as depool,
            tc.tile_pool(name="p1", bufs=2) as p1pool,
            tc.tile_pool(name="es", bufs=5) as espool,
            tc.tile_pool(name="p2", bufs=2) as p2pool,
            tc.tile_pool(name="ev", bufs=1) as epool,
            tc.tile_pool(name="ps", bufs=1, space="PSUM") as psp,
        ):
            xs_t = bigpool.tile([X, 6 * PZ, WID], f16)
            nc.sync.dma_start(out=xs_t, in_=xs.ap())
            cbs_t = bigpool.tile([X, 1], f32)
            nc.sync.dma_start(out=cbs_t, in_=cbs.ap())
            wid_t = bigpool.tile([X, NCLS * 128], f16)
            nc.sync.dma_start(out=wid_t, in_=wids.ap())
            ones_t = bigpool.tile([X, BLK, 128], f16)
            nc.gpsimd.memset(ones_t, 1.0)

            def rd(v, r0, nr, c0, ncol):
                return xs_t[:, v * PZ + r0 : v * PZ + r0 + nr, c0 : c0 + ncol]

            def lhs(pat):
                c = _CLS_IDX[pat]
                return wid_t[:, c * 128 : (c + 1) * 128]

            for blk in range(NBLK):
                R0 = blk * BLK + 2  # ext rows = xs rows [R0, R0+ER)
                p_num = psp.tile([X, BLK, 128], f32, tag="num")
                p_den = psp.tile([X, BLK, 128], f32, tag="den")
                # center tap: W = 1
                for r in range(BLK // 4):
                    nc.tensor.matmul(
                        p_num[:, 4 * r : 4 * r + 4, :], lhs((0, 0, 0)),
                        rd(0, R0 + 2 + 4 * r, 4, 4, 128),
                        start=True, stop=False,
                    )
                    nc.tensor.matmul(
                        p_den[:, 4 * r : 4 * r + 4, :], lhs((0, 0, 0)),
                        ones_t[:, 4 * r : 4 * r + 4, :],
                        start=True, stop=False,
                    )
                for pi, (dx, dy, dz) in enumerate(_PAIRS_O):
                    last = pi == len(_PAIRS_O) - 1
                    v, vn = 3 + dx, 3 - dx
                    cls = lhs((dx, abs(dy), abs(dz)))
                    # D on ext region [ER x EC], stored flat with 4-elem guards
                    # so the (dz,dy) shift below is one contiguous run.
                    d_t = depool.tile([X, 8 + ER * EC], f16)
                    dv = d_t[:, 4 : 4 + ER * EC].rearrange(
                        "p (r c) -> p r c", c=EC
                    )
                    nc.vector.tensor_sub(
                        out=dv,
                        in0=rd(0, R0, ER, 2, EC),
                        in1=rd(v, R0 + dz, ER, 2 + dy, EC),
                    )
                    # S = D^2 in place (split between DVE and ACT), E = exp(-c*S)
                    flat = d_t[:, 4 : 4 + ER * EC]
                    if pi % 7 < 2:
                        nc.vector.tensor_mul(out=flat, in0=flat, in1=flat)
                    else:
                        nc.scalar.activation(flat, flat, Sq)
                    nc.scalar.activation(flat, flat, Ex, scale=cbs_t[:, 0:1])
                    # P1 = E * A_shift on base region
                    p1_t = p1pool.tile([X, BLK, 128], f16)
                    nc.vector.tensor_mul(
                        out=p1_t,
                        in0=dv[:, 2 : 2 + BLK, 2:130],
                        in1=rd(v, R0 + 2 + dz, BLK, 4 + dy, 128),
                    )
                    # Es(j) = E(j - o). For dx=0 it is a pure free-dim shift:
                    # read E directly at offset APs. For dx>0, DMA-shift
                    # across partitions (constant flat offset; row-wrap bleed
                    # lands in pad cols, never read).
                    if dx == 0:
                        def es_sl(r0, nr):
                            return dv[:, 2 + r0 - dz : 2 + r0 - dz + nr,
                                      2 - dy : 130 - dy]
                    else:
                        es_t = espool.tile([X, BLK * EC], f16)
                        esv = es_t.rearrange("p (r c) -> p r c", c=EC)
                        off = 4 + (2 - dz) * EC - dy
                        nc.gpsimd.memset(es_t[0:dx], 0.0)
                        for a in range(0, X, 32):
                            lo = max(a, dx)
                            nc.sync.dma_start(
                                out=es_t[lo : a + 32, :],
                                in_=d_t[lo - dx : a + 32 - dx,
                                        off : off + BLK * EC],
                            )

                        def es_sl(r0, nr):
                            return esv[:, r0 : r0 + nr, 2:130]
                    # P2s = Es * A(j-o)
                    p2_t = p2pool.tile([X, BLK, 128], f16)
                    nc.vector.tensor_mul(
                        out=p2_t,
                        in0=es_sl(0, BLK),
                        in1=rd(vn, R0 + 2 - dz, BLK, 4 - dy, 128),
                    )
                    # es-independent MMs first so PE isn't queued behind the
                    # shift DMA; es-dependent MMs follow.
                    for r in range(BLK // 4):
                        sl = slice(4 * r, 4 * r + 4)
                        nc.tensor.matmul(
                            p_num[:, sl, :], cls, p1_t[:, sl, :],
                            start=False, stop=False,
                        )
                        nc.tensor.matmul(
                            p_den[:, sl, :], cls,
                            dv[:, 2 + 4 * r : 6 + 4 * r, 2:130],
                            start=False, stop=False,
                        )
                    for r in range(BLK // 4):
                        sl = slice(4 * r, 4 * r + 4)
                        nc.tensor.matmul(
                            p_num[:, sl, :], cls, p2_t[:, sl, :],
                            start=False, stop=last,
                        )
                        nc.tensor.matmul(
                            p_den[:, sl, :], cls,
                            es_sl(4 * r, 4),
                            start=False, stop=last,
                        )
                rec_t = epool.tile([X, BLK, 128], f32, tag="rec")
                nc.vector.reciprocal(out=rec_t, in_=p_den)
                o_t = epool.tile([X, BLK, 128], f32, tag="out")
                nc.vector.tensor_mul(out=o_t, in0=p_num, in1=rec_t)
                nc.sync.dma_start(
                    out=out.ap()[:, BLK * 128 * blk : BLK * 128 * (blk + 1)],
                    in_=o_t,
                )
    nc.compile()
    return nc


def _prep_core_inputs_pairs(vol, z0, big):
    """Variants for the pair kernel: index 0 = base (+BIG pads), 1..5 = x-shift
    dx=-2..2 (-BIG pads). Width 136, y_real at col 4, z_local at row 2."""
    WID = 136
    PZ = ZROWS + 4
    data = np.empty((X, PZ, 128), np.float32)
    valid = np.zeros((PZ,), bool)
    zlo = z0 - 4  # row r holds z' = r - 4
    zs_lo, zs_hi = max(0, z0 - RADIUS), min(128, z0 + ZSLAB + RADIUS)
    data[:, zs_lo - zlo : zs_hi - zlo] = vol[:, :, zs_lo:zs_hi].transpose(0, 2, 1)
    valid[zs_lo - zlo : zs_hi - zlo] = True

    xs = np.empty((X, 6, PZ, WID), np.float32)
    xs[:, 0] = big
    xs[:, 0, valid, 4:132] = data[:, valid]
    for dx in range(-RADIUS, RADIUS + 1):
        vi = 3 + dx
        xs[:, vi] = -big
        if dx >= 0:
            xs[: X - dx, vi, valid, 4:132] = data[dx:][:, valid]
        else:
            xs[-dx:, vi, valid, 4:132] = data[: X + dx][:, valid]
    return xs.astype(np.float16).reshape(X, 6 * PZ, WID)


def _prep_core_inputs(vol, z0, big, np_x, mode):
    """vol: (128,128,128) f32 volume (x,y,z) for one batch. Returns xs array."""
    nv = 10 if mode == "f16" else 5
    wid = 136 if mode == "f16" else 132
    slab = np.full((X, ZROWS, 130), big, np.float32)
    zlo = z0 - RADIUS
    zs_lo, zs_hi = max(0, zlo), min(128, z0 + ZSLAB + RADIUS)
    # rows (z_local) x cols (y)
    slab[:, zs_lo - zlo : zs_hi - zlo, 2:130] = vol[:, :, zs_lo:zs_hi].transpose(0, 2, 1)
    xs = np.full((X, nv, ZROWS, wid), big, np_x)
    for dx in range(-RADIUS, RADIUS + 1):
        var = np.full((X, ZROWS, 130), big, np.float32)
        if dx >= 0:
            var[: X - dx] = slab[dx:]
        else:
            var[-dx:] = slab[: X + dx]
        if mode == "f16":
            v = (dx + RADIUS) * 2
            xs[:, v, :, 0:130] = var  # parity 0: y_real at col 2
            xs[:, v + 1, :, 1:131] = var  # parity 1: y_real at col 3
        else:
            xs[:, dx + RADIUS, :, 0:130] = var
    return xs.reshape(X, nv * ZROWS, wid)


def kernel(input_img, sigma_x, sigma_y, sigma_z, color_sigma):
    global LAST_RESULTS
    img = np.asarray(input_img, dtype=np.float32)
    B = img.shape[0]
    sx = float(np.asarray(sigma_x))
    sy = float(np.asarray(sigma_y))
    sz = float(np.asarray(sigma_z))
    cs = float(np.asarray(color_sigma))
    c = 1.0 / (2.0 * cs * cs)

    xmax = float(np.abs(img).max())
    big = xmax + np.sqrt(95.0 / c)

    if PAIRS and MODE == "f16":
        key = "pairs"
        if key not in _PROG_CACHE:
            _PROG_CACHE[key] = _build_program_pairs()
        nc = _PROG_CACHE[key]
        cbsv = np.full((X, 1), -c, np.float32)
        eye = np.eye(128, dtype=np.float32)
        widv = np.empty((X, NCLS, 128), np.float32)
        for i, (px, py, pz) in enumerate(_CLS_PATS):
            wsp = np.exp(-(px * px / (2 * sx * sx) + py * py / (2 * sy * sy)
                           + pz * pz / (2 * sz * sz)))
            widv[:, i, :] = wsp * eye
        widv = widv.astype(np.float16).reshape(X, NCLS * 128)
        in_maps = []
        for core in range(8):
            b, q = divmod(core, 4)
            xsv = _prep_core_inputs_pairs(img[b, 0], q * ZSLAB, big)
            in_maps.append({"xs": xsv, "cbs": cbsv, "wids": widv})
    else:
        if MODE not in _PROG_CACHE:
            _PROG_CACHE[MODE] = _build_program(MODE)
        nc, np_x = _PROG_CACHE[MODE]

        # per-tap log spatial weights and exp scale
        cbv = np.zeros((X, NTAPS + 1), np.float32)
        for k, (dx, dy, dz) in enumerate(_TAPS):
            cbv[:, k] = -(dx * dx / (2 * sx * sx) + dy * dy / (2 * sy * sy)
                          + dz * dz / (2 * sz * sz))
        cbv[:, NTAPS] = -c

        eye = np.eye(X, dtype=np.float32)
        in_maps = []
        for core in range(8):
            b, q = divmod(core, 4)
            xs = _prep_core_inputs(img[b, 0], q * ZSLAB, big, np_x, MODE)
            in_maps.append({"xs": xs, "cb": cbv, "ident": eye})

    res = bass_utils.run_bass_kernel_spmd(
        nc, in_maps, core_ids=list(range(8)), trace=TRACE
    )
    LAST_RESULTS = res

    outv = np.empty_like(img)
    for core in range(8):
        b, q = divmod(core, 4)
        o = res.results[core]["out"].reshape(X, ZSLAB, 128)  # (x, z_local, y)
        outv[b, 0, :, :, q * ZSLAB : (q + 1) * ZSLAB] = o.transpose(0, 2, 1)
    return outv
